# revision 6
# baseline (speedup 1.0000x reference)
"""Trainium2 Bass kernel for segmented-LoRA linear (nn_Linear_73959336837249).

Math: out = x @ W.T + scale_g * ((x_g @ A_g.T) @ B_g.T), where the 16384
tokens form 4 contiguous segments of 4096, one adapter per segment.

Strategy:
  * Fold the LoRA update into the base weight per adapter on the host:
        Weff_g = W + s_g * B_g @ A_g        (exact algebraic identity)
    so each token segment needs a single dense matmul x_g @ Weff_g.T.
  * Shard tokens across the 8 NeuronCores (2048 tokens/core); each core's
    token range lives entirely inside one adapter segment, so each core
    gets exactly one [2048, 2048] effective weight.
  * On device: one big [2048 x 2048] @ [2048 x 2048] matmul per core,
    K-tiled over PSUM. Inputs are fed as bf16 (rel err ~2e-3, well inside
    the 2e-2 gate); accumulation is fp32 in PSUM.

Schedule (from perfetto trace analysis of the naive version):
  * PE warm-up: a few matmuls on zeroed scratch right after the engine
    preamble, so the HAM clock-gate reaches 8/8 (~2.4 GHz) before real
    data arrives and the PE never runs real work at 1.2 GHz.
  * The W stream (8 MiB, ~23 us at full DMA bw) is issued per-k-tile so
    completion granularity is fine; the first k-tile goes in 512-col
    pieces so the very first matmul starts as early as possible.
  * Token tiles t=0,1 are processed as an interleaved PAIR with the
    k-loop outermost, using all 8 PSUM banks. That halves the PE's
    W-consumption rate during the phase where W is still streaming in,
    eliminating the stalls the naive one-tile-at-a-time order has.
  * Tiles t=2..14 run one at a time (4+4 PSUM banks double-buffered),
    back-to-back matmuls at the 512-col streaming floor.
  * Tail tile t=15 runs o-outer / k-inner so each PSUM bank completes
    (and is copied out + DMA'd) while later banks still compute; the
    final out chunks ride different engine queues so their triggers
    don't serialize.

Self-contained: hardcodes all shapes; no file I/O.
"""

import numpy as np

# Problem shapes (hardcoded per contest contract)
N_ADAPTERS = 4
RANK = 16
D_IN = 2048
D_OUT = 2048
TOKENS = 16384
N_CORES = 8

T_LOC = TOKENS // N_CORES  # 2048 tokens per core
P = 128                    # partitions
KT = D_IN // P             # 16 contraction tiles
TT = T_LOC // P            # 16 token tiles per core
ON = 512                   # output-column tile (one PSUM bank of fp32)
NO = D_OUT // ON           # 4 o-tiles

N_WARM = 7                 # PE warm-up matmuls (N=512 each, ~3 us at 1.2 GHz)

IN_DTYPE = "bf16"

_NC = {}


def _np_in_dtype():
    import ml_dtypes

    return np.dtype(ml_dtypes.bfloat16)


def _build_nc():
    import concourse.mybir as mybir
    import concourse.tile as tile
    from concourse import bacc

    fp32 = mybir.dt.float32
    idt = mybir.dt.bfloat16

    nc = bacc.Bacc(None, target_bir_lowering=False)

    # xt[t, p, k*128+j] = x_tok[t*128+j, k*128+p]  (token-tile-major, d on partitions)
    xt = nc.dram_tensor("xt", [TT, P, KT * P], idt, kind="ExternalInput")
    # wt[p, k*D_OUT + o] = Weff.T[k*128+p, o]  (matches the SBUF-resident layout)
    wt = nc.dram_tensor("wt", [P, KT * D_OUT], idt, kind="ExternalInput")
    out = nc.dram_tensor("out", [T_LOC, D_OUT], fp32, kind="ExternalOutput")

    def wsl(k, o):  # SBUF W slice for (k, o)
        return (k * D_OUT + o * ON, k * D_OUT + (o + 1) * ON)

    with tile.TileContext(nc) as tc:
        with (
            tc.tile_pool(name="wpool", bufs=1) as wpool,
            tc.tile_pool(name="xpool", bufs=3) as xpool,
            tc.tile_pool(name="opool", bufs=3) as opool,
            tc.tile_pool(name="spool", bufs=1) as spool,
            tc.tile_pool(name="pspool", bufs=2, space="PSUM") as pspool,
        ):
            # Scratch for PE warm-up. Memset on Vector: it reaches "main"
            # earliest of all engines (~5.9 us) and the DVE does bf16 SBUF
            # memsets at 4x rate, so warm-up matmuls can start ~6.4 us.
            scr = spool.tile([P, ON], idt, tag="scr", name="scratch")
            nc.vector.memset(scr[:], 0)

            x_tiles = {}
            for t in (0, 1):
                x_tiles[t] = xpool.tile([P, KT * P], idt, tag="x", name=f"x_{t}")
            wall = wpool.tile([P, KT * D_OUT], idt, tag="w", name="wall")

            # First-data triggers, spread across the two HWDGE engines by
            # their observed "main" start times (scalar ~6.0 us, sync ~6.8)
            # and ordered so the DMA queues serve exactly what the pair
            # phase consumes next: x0/x1 k0-slices + w0 pieces, then W
            # per-k-tile. The bulky x tails (x0p2/x1p2) are emitted AFTER
            # k3 so their bytes cannot delay the W k-tiles in the queues.
            w0 = [wsl(0, o) for o in range(NO)]
            nc.scalar.dma_start(x_tiles[0][:, : 4 * P], xt[0, :, : 4 * P])
            nc.scalar.dma_start(wall[:, w0[0][0] : w0[0][1]], wt[:, w0[0][0] : w0[0][1]])
            nc.scalar.dma_start(x_tiles[1][:, : 4 * P], xt[1, :, : 4 * P])
            nc.scalar.dma_start(wall[:, w0[1][0] : w0[1][1]], wt[:, w0[1][0] : w0[1][1]])
            nc.sync.dma_start(wall[:, w0[2][0] : w0[2][1]], wt[:, w0[2][0] : w0[2][1]])
            nc.sync.dma_start(wall[:, w0[3][0] : w0[3][1]], wt[:, w0[3][0] : w0[3][1]])
            for k in range(1, KT):
                nc.sync.dma_start(
                    wall[:, k * D_OUT : (k + 1) * D_OUT],
                    wt[:, k * D_OUT : (k + 1) * D_OUT],
                )
                if k == 3:
                    nc.sync.dma_start(x_tiles[0][:, 4 * P :], xt[0, :, 4 * P :])
                    nc.sync.dma_start(x_tiles[1][:, 4 * P :], xt[1, :, 4 * P :])

            # PSUM banks for the pair: all 8 banks (2 generations x 4 tags).
            ps = {}
            for t in (0, 1):
                for o in range(NO):
                    ps[(t, o)] = pspool.tile(
                        [P, ON], fp32, tag=f"ps{o}", name=f"ps_{t}_{o}"
                    )

            # PE warm-up: garbage matmuls on zeroed scratch, overwritten by
            # the real k=0 (start=True) matmuls later.
            for i in range(N_WARM):
                nc.tensor.matmul(
                    ps[(0, i % NO)][:], scr[:, :P], scr[:], start=True, stop=True
                )

            # ---- pair phase: t=0,1 interleaved, k outermost ----
            # o-outer / t-inner within each k so the k=0 matmuls consume the
            # four w0 pieces in their DMA arrival order.
            for k in range(KT):
                for o in range(NO):
                    a, b = wsl(k, o)
                    for t in (0, 1):
                        lhsT = x_tiles[t][:, k * P : (k + 1) * P]
                        nc.tensor.matmul(
                            ps[(t, o)][:],
                            lhsT,
                            wall[:, a:b],
                            start=(k == 0),
                            stop=(k == KT - 1),
                        )
            for t in (0, 1):
                o_t = opool.tile([P, D_OUT], fp32, tag="o", name=f"o_{t}")
                for o in range(NO):
                    nc.vector.tensor_copy(o_t[:, o * ON : (o + 1) * ON], ps[(t, o)][:])
                nc.sync.dma_start(
                    out[t * P : (t + 1) * P, : D_OUT // 2], o_t[:, : D_OUT // 2]
                )
                nc.sync.dma_start(
                    out[t * P : (t + 1) * P, D_OUT // 2 :], o_t[:, D_OUT // 2 :]
                )

            # x_2 rides the sync queue so its bytes arrive AFTER the W
            # stream in the DMA queues (x_3.. are gated by pool slots).
            x_tiles[2] = xpool.tile([P, KT * P], idt, tag="x", name="x_2")
            nc.sync.dma_start(x_tiles[2][:], xt[2])

            # ---- steady phase: t=2..14, one tile at a time ----
            for t in range(2, TT - 1):
                if t not in x_tiles:
                    x_tiles[t] = xpool.tile([P, KT * P], idt, tag="x", name=f"x_{t}")
                    nc.scalar.dma_start(x_tiles[t][:], xt[t])
                pst = [
                    pspool.tile([P, ON], fp32, tag=f"ps{o}", name=f"ps_{t}_{o}")
                    for o in range(NO)
                ]
                for k in range(KT):
                    lhsT = x_tiles[t][:, k * P : (k + 1) * P]
                    for o in range(NO):
                        a, b = wsl(k, o)
                        nc.tensor.matmul(
                            pst[o][:],
                            lhsT,
                            wall[:, a:b],
                            start=(k == 0),
                            stop=(k == KT - 1),
                        )
                o_t = opool.tile([P, D_OUT], fp32, tag="o", name=f"o_{t}")
                for o in range(NO):
                    nc.vector.tensor_copy(o_t[:, o * ON : (o + 1) * ON], pst[o][:])
                nc.sync.dma_start(
                    out[t * P : (t + 1) * P, : D_OUT // 2], o_t[:, : D_OUT // 2]
                )
                nc.sync.dma_start(
                    out[t * P : (t + 1) * P, D_OUT // 2 :], o_t[:, D_OUT // 2 :]
                )

            # ---- tail tile t=15: o-outer so banks complete progressively ----
            t = TT - 1
            x_tiles[t] = xpool.tile([P, KT * P], idt, tag="x", name=f"x_{t}")
            nc.scalar.dma_start(x_tiles[t][:], xt[t])
            ps15 = [
                pspool.tile([P, ON], fp32, tag=f"ps{o}", name=f"ps_{t}_{o}")
                for o in range(NO)
            ]
            o_t = opool.tile([P, D_OUT], fp32, tag="o", name=f"o_{t}")
            out_eng = [nc.sync, nc.scalar, nc.sync, nc.gpsimd]
            for o in range(NO):
                for k in range(KT):
                    lhsT = x_tiles[t][:, k * P : (k + 1) * P]
                    a, b = wsl(k, o)
                    nc.tensor.matmul(
                        ps15[o][:],
                        lhsT,
                        wall[:, a:b],
                        start=(k == 0),
                        stop=(k == KT - 1),
                    )
                nc.vector.tensor_copy(o_t[:, o * ON : (o + 1) * ON], ps15[o][:])
                out_eng[o].dma_start(
                    out[t * P : (t + 1) * P, o * ON : (o + 1) * ON],
                    o_t[:, o * ON : (o + 1) * ON],
                )

    nc.compile()
    return nc


def _get_nc():
    if IN_DTYPE not in _NC:
        _NC[IN_DTYPE] = _build_nc()
    return _NC[IN_DTYPE]


def _prep_inputs(inputs):
    x = np.ascontiguousarray(np.asarray(inputs["x"], dtype=np.float32))
    W = np.asarray(inputs["W"], dtype=np.float32)
    lora_a = np.asarray(inputs["lora_a"], dtype=np.float32)
    lora_b = np.asarray(inputs["lora_b"], dtype=np.float32)
    scalings = np.asarray(inputs["scalings"], dtype=np.float32)
    idt = _np_in_dtype()

    # Fold LoRA into the transposed effective weight per adapter:
    # Weff.T = W.T + s * A.T @ B.T  -> [d_in, d_out],
    # laid out as [P, KT*D_OUT] with wt[p, k*D_OUT+o] = Weff.T[k*128+p, o].
    wts = []
    for g in range(N_ADAPTERS):
        weff_t = W.T + scalings[g] * (lora_a[g].T @ lora_b[g].T)
        wts.append(
            np.ascontiguousarray(
                weff_t.reshape(KT, P, D_OUT).transpose(1, 0, 2).astype(idt)
            ).reshape(P, KT * D_OUT)
        )

    in_maps = []
    for c in range(N_CORES):
        xs = x[c * T_LOC : (c + 1) * T_LOC]  # [2048 tok, 2048 d]
        # [t, j, k, p] -> [t, p, k, j] -> [TT, 128, KT*128]
        xtl = np.ascontiguousarray(
            xs.reshape(TT, P, KT, P).transpose(0, 3, 2, 1).astype(idt)
        ).reshape(TT, P, KT * P)
        in_maps.append({"xt": xtl, "wt": wts[c * T_LOC // (TOKENS // N_ADAPTERS)]})
    return in_maps


def _run(inputs, trace=False, **kwargs):
    from concourse.bass_utils import run_bass_kernel_spmd

    nc = _get_nc()
    in_maps = _prep_inputs(inputs)
    res = run_bass_kernel_spmd(
        nc, in_maps, core_ids=list(range(N_CORES)), trace=trace, **kwargs
    )
    out = np.concatenate([r["out"] for r in res.results], axis=0)
    return out, res


def kernel(**inputs):
    out, _ = _run(inputs, trace=False)
    return out


# revision 10
# speedup vs baseline: 1.0140x; 1.0140x over previous
"""Trainium2 Bass kernel for segmented-LoRA linear (nn_Linear_73959336837249).

Math: out = x @ W.T + scale_g * ((x_g @ A_g.T) @ B_g.T), where the 16384
tokens form 4 contiguous segments of 4096, one adapter per segment.

Strategy:
  * Fold the LoRA update into the base weight per adapter on the host:
        Weff_g = W + s_g * B_g @ A_g        (exact algebraic identity)
    so each token segment needs a single dense matmul x_g @ Weff_g.T.
  * Shard tokens across the 8 NeuronCores (2048 tokens/core); each core's
    token range lives entirely inside one adapter segment, so each core
    gets exactly one [2048, 2048] effective weight.
  * On device: one big [2048 x 2048] @ [2048 x 2048] matmul per core,
    K-tiled over PSUM. Inputs are fed as bf16 (rel err ~2e-3, well inside
    the 2e-2 gate); accumulation is fp32 in PSUM.

Schedule (from perfetto trace analysis of the naive version):
  * PE warm-up: a few matmuls on zeroed scratch right after the engine
    preamble, so the HAM clock-gate reaches 8/8 (~2.4 GHz) before real
    data arrives and the PE never runs real work at 1.2 GHz.
  * The W stream (8 MiB, ~23 us at full DMA bw) is issued per-k-tile so
    completion granularity is fine; the first k-tile goes in 512-col
    pieces so the very first matmul starts as early as possible.
  * Token tiles t=0,1 are processed as an interleaved PAIR with the
    k-loop outermost, using all 8 PSUM banks. That halves the PE's
    W-consumption rate during the phase where W is still streaming in,
    eliminating the stalls the naive one-tile-at-a-time order has.
  * Tiles t=2..14 run one at a time (4+4 PSUM banks double-buffered),
    back-to-back matmuls at the 512-col streaming floor.
  * Tail tile t=15 runs o-outer / k-inner so each PSUM bank completes
    (and is copied out + DMA'd) while later banks still compute; the
    final out chunks ride different engine queues so their triggers
    don't serialize.

Self-contained: hardcodes all shapes; no file I/O.
"""

import numpy as np

# Problem shapes (hardcoded per contest contract)
N_ADAPTERS = 4
RANK = 16
D_IN = 2048
D_OUT = 2048
TOKENS = 16384
N_CORES = 8

T_LOC = TOKENS // N_CORES  # 2048 tokens per core
P = 128                    # partitions
KT = D_IN // P             # 16 contraction tiles
TT = T_LOC // P            # 16 token tiles per core
ON = 512                   # output-column tile (one PSUM bank of fp32)
NO = D_OUT // ON           # 4 o-tiles

N_WARM = 4                 # PE warm-up matmuls (N=512 each, ~1.7 us at 1.2 GHz)

IN_DTYPE = "bf16"

_NC = {}


def _np_in_dtype():
    import ml_dtypes

    return np.dtype(ml_dtypes.bfloat16)


def _build_nc():
    import concourse.mybir as mybir
    import concourse.tile as tile
    from concourse import bacc

    fp32 = mybir.dt.float32
    idt = mybir.dt.bfloat16

    nc = bacc.Bacc(None, target_bir_lowering=False)

    # xt[t, p, k*128+j] = x_tok[t*128+j, k*128+p]  (token-tile-major, d on partitions)
    xt = nc.dram_tensor("xt", [TT, P, KT * P], idt, kind="ExternalInput")
    # wt[p, k*D_OUT + o] = Weff.T[k*128+p, o]  (matches the SBUF-resident layout)
    wt = nc.dram_tensor("wt", [P, KT * D_OUT], idt, kind="ExternalInput")
    out = nc.dram_tensor("out", [T_LOC, D_OUT], fp32, kind="ExternalOutput")

    def wsl(k, o):  # SBUF W slice for (k, o)
        return (k * D_OUT + o * ON, k * D_OUT + (o + 1) * ON)

    with tile.TileContext(nc) as tc:
        with (
            tc.tile_pool(name="wpool", bufs=1) as wpool,
            tc.tile_pool(name="xpool", bufs=3) as xpool,
            tc.tile_pool(name="opool", bufs=3) as opool,
            tc.tile_pool(name="spool", bufs=1) as spool,
            tc.tile_pool(name="pspool", bufs=2, space="PSUM") as pspool,
        ):
            # Scratch for PE warm-up. Memset on Vector: it reaches "main"
            # earliest of all engines (~5.9 us) and the DVE does bf16 SBUF
            # memsets at 4x rate, so warm-up matmuls can start ~6.4 us.
            scr = spool.tile([P, ON], idt, tag="scr", name="scratch")
            nc.vector.memset(scr[:], 0)

            x_tiles = {}
            for t in (0, 1):
                x_tiles[t] = xpool.tile([P, KT * P], idt, tag="x", name=f"x_{t}")
            wall = wpool.tile([P, KT * D_OUT], idt, tag="w", name="wall")

            # All latency-critical early transfers ride the SYNC hwdge ring,
            # in exact consumption order. (The two hwdge rings share the 16
            # DMA engines and the busy ring starves the other ~8:1, so
            # splitting early pieces across rings makes them arrive LATE.)
            # The x tails are split per k-range so each piece lands just
            # before the pair loop consumes it, without delaying W k-tiles.
            w0 = [wsl(0, o) for o in range(NO)]

            def wdma(k):
                nc.sync.dma_start(
                    wall[:, k * D_OUT : (k + 1) * D_OUT],
                    wt[:, k * D_OUT : (k + 1) * D_OUT],
                )

            nc.sync.dma_start(x_tiles[0][:, : 4 * P], xt[0, :, : 4 * P])
            nc.sync.dma_start(wall[:, w0[0][0] : w0[0][1]], wt[:, w0[0][0] : w0[0][1]])
            nc.sync.dma_start(x_tiles[1][:, : 4 * P], xt[1, :, : 4 * P])
            for o in (1, 2, 3):
                nc.sync.dma_start(wall[:, w0[o][0] : w0[o][1]], wt[:, w0[o][0] : w0[o][1]])
            for k in (1, 2, 3):
                wdma(k)
            nc.sync.dma_start(x_tiles[0][:, 4 * P : 8 * P], xt[0, :, 4 * P : 8 * P])
            nc.sync.dma_start(x_tiles[1][:, 4 * P : 8 * P], xt[1, :, 4 * P : 8 * P])
            for k in (4, 5, 6, 7):
                wdma(k)
            nc.sync.dma_start(x_tiles[0][:, 8 * P :], xt[0, :, 8 * P :])
            nc.sync.dma_start(x_tiles[1][:, 8 * P :], xt[1, :, 8 * P :])
            for k in (8, 9, 10, 11, 12, 13):
                wdma(k)
            # x_2 slots in before the last two W tiles (it isn't needed
            # until the pair completes, and this keeps k14/k15 on time).
            x_tiles[2] = xpool.tile([P, KT * P], idt, tag="x", name="x_2")
            nc.sync.dma_start(x_tiles[2][:], xt[2])
            wdma(14)
            wdma(15)

            # PSUM banks for the pair: all 8 banks (2 generations x 4 tags).
            ps = {}
            for t in (0, 1):
                for o in range(NO):
                    ps[(t, o)] = pspool.tile(
                        [P, ON], fp32, tag=f"ps{o}", name=f"ps_{t}_{o}"
                    )

            # PE warm-up: garbage matmuls on zeroed scratch, overwritten by
            # the real k=0 (start=True) matmuls later.
            for i in range(N_WARM):
                nc.tensor.matmul(
                    ps[(0, i % NO)][:], scr[:, :P], scr[:], start=True, stop=True
                )

            # ---- pair phase: t=0,1 interleaved, k outermost ----
            # o-outer / t-inner within each k so the k=0 matmuls consume the
            # four w0 pieces in their DMA arrival order.
            for k in range(KT):
                for o in range(NO):
                    a, b = wsl(k, o)
                    for t in (0, 1):
                        lhsT = x_tiles[t][:, k * P : (k + 1) * P]
                        nc.tensor.matmul(
                            ps[(t, o)][:],
                            lhsT,
                            wall[:, a:b],
                            start=(k == 0),
                            stop=(k == KT - 1),
                        )
            for t in (0, 1):
                o_t = opool.tile([P, D_OUT], fp32, tag="o", name=f"o_{t}")
                for o in range(NO):
                    nc.vector.tensor_copy(o_t[:, o * ON : (o + 1) * ON], ps[(t, o)][:])
                nc.sync.dma_start(
                    out[t * P : (t + 1) * P, : D_OUT // 2], o_t[:, : D_OUT // 2]
                )
                nc.sync.dma_start(
                    out[t * P : (t + 1) * P, D_OUT // 2 :], o_t[:, D_OUT // 2 :]
                )

            # ---- steady phase: t=2..14, one tile at a time ----
            for t in range(2, TT - 1):
                if t not in x_tiles:
                    x_tiles[t] = xpool.tile([P, KT * P], idt, tag="x", name=f"x_{t}")
                    nc.scalar.dma_start(x_tiles[t][:], xt[t])
                pst = [
                    pspool.tile([P, ON], fp32, tag=f"ps{o}", name=f"ps_{t}_{o}")
                    for o in range(NO)
                ]
                for k in range(KT):
                    lhsT = x_tiles[t][:, k * P : (k + 1) * P]
                    for o in range(NO):
                        a, b = wsl(k, o)
                        nc.tensor.matmul(
                            pst[o][:],
                            lhsT,
                            wall[:, a:b],
                            start=(k == 0),
                            stop=(k == KT - 1),
                        )
                o_t = opool.tile([P, D_OUT], fp32, tag="o", name=f"o_{t}")
                for o in range(NO):
                    nc.vector.tensor_copy(o_t[:, o * ON : (o + 1) * ON], pst[o][:])
                nc.sync.dma_start(
                    out[t * P : (t + 1) * P, : D_OUT // 2], o_t[:, : D_OUT // 2]
                )
                nc.sync.dma_start(
                    out[t * P : (t + 1) * P, D_OUT // 2 :], o_t[:, D_OUT // 2 :]
                )

            # ---- tail tile t=15: o-outer so banks complete progressively ----
            t = TT - 1
            x_tiles[t] = xpool.tile([P, KT * P], idt, tag="x", name=f"x_{t}")
            nc.scalar.dma_start(x_tiles[t][:], xt[t])
            ps15 = [
                pspool.tile([P, ON], fp32, tag=f"ps{o}", name=f"ps_{t}_{o}")
                for o in range(NO)
            ]
            o_t = opool.tile([P, D_OUT], fp32, tag="o", name=f"o_{t}")
            out_eng = [nc.sync, nc.scalar, nc.sync, nc.gpsimd]
            for o in range(NO):
                for k in range(KT):
                    lhsT = x_tiles[t][:, k * P : (k + 1) * P]
                    a, b = wsl(k, o)
                    nc.tensor.matmul(
                        ps15[o][:],
                        lhsT,
                        wall[:, a:b],
                        start=(k == 0),
                        stop=(k == KT - 1),
                    )
                nc.vector.tensor_copy(o_t[:, o * ON : (o + 1) * ON], ps15[o][:])
                out_eng[o].dma_start(
                    out[t * P : (t + 1) * P, o * ON : (o + 1) * ON],
                    o_t[:, o * ON : (o + 1) * ON],
                )

    nc.compile()
    return nc


def _get_nc():
    if IN_DTYPE not in _NC:
        _NC[IN_DTYPE] = _build_nc()
    return _NC[IN_DTYPE]


def _prep_inputs(inputs):
    x = np.ascontiguousarray(np.asarray(inputs["x"], dtype=np.float32))
    W = np.asarray(inputs["W"], dtype=np.float32)
    lora_a = np.asarray(inputs["lora_a"], dtype=np.float32)
    lora_b = np.asarray(inputs["lora_b"], dtype=np.float32)
    scalings = np.asarray(inputs["scalings"], dtype=np.float32)
    idt = _np_in_dtype()

    # Fold LoRA into the transposed effective weight per adapter:
    # Weff.T = W.T + s * A.T @ B.T  -> [d_in, d_out],
    # laid out as [P, KT*D_OUT] with wt[p, k*D_OUT+o] = Weff.T[k*128+p, o].
    wts = []
    for g in range(N_ADAPTERS):
        weff_t = W.T + scalings[g] * (lora_a[g].T @ lora_b[g].T)
        wts.append(
            np.ascontiguousarray(
                weff_t.reshape(KT, P, D_OUT).transpose(1, 0, 2).astype(idt)
            ).reshape(P, KT * D_OUT)
        )

    in_maps = []
    for c in range(N_CORES):
        xs = x[c * T_LOC : (c + 1) * T_LOC]  # [2048 tok, 2048 d]
        # [t, j, k, p] -> [t, p, k, j] -> [TT, 128, KT*128]
        xtl = np.ascontiguousarray(
            xs.reshape(TT, P, KT, P).transpose(0, 3, 2, 1).astype(idt)
        ).reshape(TT, P, KT * P)
        in_maps.append({"xt": xtl, "wt": wts[c * T_LOC // (TOKENS // N_ADAPTERS)]})
    return in_maps


def _run(inputs, trace=False, **kwargs):
    from concourse.bass_utils import run_bass_kernel_spmd

    nc = _get_nc()
    in_maps = _prep_inputs(inputs)
    res = run_bass_kernel_spmd(
        nc, in_maps, core_ids=list(range(N_CORES)), trace=trace, **kwargs
    )
    out = np.concatenate([r["out"] for r in res.results], axis=0)
    return out, res


def kernel(**inputs):
    out, _ = _run(inputs, trace=False)
    return out


# revision 15
# speedup vs baseline: 1.0249x; 1.0107x over previous
"""Trainium2 Bass kernel for segmented-LoRA linear (nn_Linear_73959336837249).

Math: out = x @ W.T + scale_g * ((x_g @ A_g.T) @ B_g.T), where the 16384
tokens form 4 contiguous segments of 4096, one adapter per segment.

Strategy:
  * Fold the LoRA update into the base weight per adapter on the host:
        Weff_g = W + s_g * B_g @ A_g        (exact algebraic identity)
    so each token segment needs a single dense matmul x_g @ Weff_g.T.
  * Shard tokens across the 8 NeuronCores (2048 tokens/core); each core's
    token range lives entirely inside one adapter segment, so each core
    gets exactly one [2048, 2048] effective weight.
  * On device: one big [2048 x 2048] @ [2048 x 2048] matmul per core,
    K-tiled over PSUM. Inputs are fed as bf16 (rel err ~2e-3, well inside
    the 2e-2 gate); accumulation is fp32 in PSUM.

Schedule (from perfetto trace analysis of the naive version):
  * PE warm-up: a few matmuls on zeroed scratch right after the engine
    preamble, so the HAM clock-gate reaches 8/8 (~2.4 GHz) before real
    data arrives and the PE never runs real work at 1.2 GHz.
  * The W stream (8 MiB, ~23 us at full DMA bw) is issued per-k-tile so
    completion granularity is fine; the first k-tile goes in 512-col
    pieces so the very first matmul starts as early as possible.
  * Token tiles t=0,1 are processed as an interleaved PAIR with the
    k-loop outermost, using all 8 PSUM banks. That halves the PE's
    W-consumption rate during the phase where W is still streaming in,
    eliminating the stalls the naive one-tile-at-a-time order has.
  * Tiles t=2..14 run one at a time (4+4 PSUM banks double-buffered),
    back-to-back matmuls at the 512-col streaming floor.
  * Tail tile t=15 runs o-outer / k-inner so each PSUM bank completes
    (and is copied out + DMA'd) while later banks still compute; the
    final out chunks ride different engine queues so their triggers
    don't serialize.

Self-contained: hardcodes all shapes; no file I/O.
"""

import numpy as np

# Problem shapes (hardcoded per contest contract)
N_ADAPTERS = 4
RANK = 16
D_IN = 2048
D_OUT = 2048
TOKENS = 16384
N_CORES = 8

T_LOC = TOKENS // N_CORES  # 2048 tokens per core
P = 128                    # partitions
KT = D_IN // P             # 16 contraction tiles
TT = T_LOC // P            # 16 token tiles per core
ON = 512                   # output-column tile (one PSUM bank of fp32)
NO = D_OUT // ON           # 4 o-tiles

N_WARM = 18                # N=128 warm-up matmuls before the first real matmul
N_WARM2 = 4                # extra fills before t1's first matmul (its bank only)

IN_DTYPE = "bf16"

_NC = {}


def _np_in_dtype():
    import ml_dtypes

    return np.dtype(ml_dtypes.bfloat16)


def _build_nc():
    import concourse.mybir as mybir
    import concourse.tile as tile
    from concourse import bacc

    fp32 = mybir.dt.float32
    idt = mybir.dt.bfloat16

    nc = bacc.Bacc(None, target_bir_lowering=False)

    # xt[t, p, k*128+j] = x_tok[t*128+j, k*128+p]  (token-tile-major, d on partitions)
    xt = nc.dram_tensor("xt", [TT, P, KT * P], idt, kind="ExternalInput")
    # wt[p, k*D_OUT + o] = Weff.T[k*128+p, o]  (matches the SBUF-resident layout)
    wt = nc.dram_tensor("wt", [P, KT * D_OUT], idt, kind="ExternalInput")
    out = nc.dram_tensor("out", [T_LOC, D_OUT], fp32, kind="ExternalOutput")

    def wsl(k, o):  # SBUF W slice for (k, o)
        return (k * D_OUT + o * ON, k * D_OUT + (o + 1) * ON)

    with tile.TileContext(nc) as tc:
        with (
            tc.tile_pool(name="wpool", bufs=1) as wpool,
            tc.tile_pool(name="xpool", bufs=3) as xpool,
            tc.tile_pool(name="opool", bufs=3) as opool,
            tc.tile_pool(name="spool", bufs=1) as spool,
            tc.tile_pool(name="pspool", bufs=2, space="PSUM") as pspool,
        ):
            # Scratch for PE warm-up. Memset on Vector: it reaches "main"
            # earliest of all engines (~5.9 us) and the DVE does bf16 SBUF
            # memsets at 4x rate, so warm-up matmuls can start ~6.4 us.
            scr = spool.tile([P, ON], idt, tag="scr", name="scratch")
            nc.vector.memset(scr[:], 0)

            x_tiles = {}
            for t in (0, 1):
                x_tiles[t] = xpool.tile([P, KT * P], idt, tag="x", name=f"x_{t}")
            wall = wpool.tile([P, KT * D_OUT], idt, tag="w", name="wall")

            # All latency-critical early transfers ride the SYNC hwdge ring,
            # in exact consumption order. (The two hwdge rings share the 16
            # DMA engines and the busy ring starves the other ~8:1, so
            # splitting early pieces across rings makes them arrive LATE.)
            # The x tails are split per k-range so each piece lands just
            # before the pair loop consumes it, without delaying W k-tiles.
            def wdma(k):
                nc.sync.dma_start(
                    wall[:, k * D_OUT : (k + 1) * D_OUT],
                    wt[:, k * D_OUT : (k + 1) * D_OUT],
                )

            # k0's W goes in 3 pieces: a small first piece so the first real
            # matmul starts ASAP, then two merged pieces (fewer early
            # triggers keep the DMA queues fed while they ramp).
            nc.sync.dma_start(x_tiles[0][:, : 4 * P], xt[0, :, : 4 * P])
            nc.sync.dma_start(wall[:, 0:512], wt[:, 0:512])
            nc.sync.dma_start(x_tiles[1][:, : 4 * P], xt[1, :, : 4 * P])
            nc.sync.dma_start(wall[:, 512:1536], wt[:, 512:1536])
            nc.sync.dma_start(wall[:, 1536:2048], wt[:, 1536:2048])
            for k in (1, 2, 3):
                wdma(k)
            nc.sync.dma_start(x_tiles[0][:, 4 * P : 8 * P], xt[0, :, 4 * P : 8 * P])
            nc.sync.dma_start(x_tiles[1][:, 4 * P : 8 * P], xt[1, :, 4 * P : 8 * P])
            for k in (4, 5, 6, 7):
                wdma(k)
            nc.sync.dma_start(x_tiles[0][:, 8 * P :], xt[0, :, 8 * P :])
            nc.sync.dma_start(x_tiles[1][:, 8 * P :], xt[1, :, 8 * P :])
            for k in (8, 9, 10, 11, 12, 13):
                wdma(k)
            # x_2 slots in before the last two W tiles (it isn't needed
            # until the pair completes, and this keeps k14/k15 on time).
            x_tiles[2] = xpool.tile([P, KT * P], idt, tag="x", name="x_2")
            nc.sync.dma_start(x_tiles[2][:], xt[2])
            wdma(14)
            wdma(15)

            # PSUM banks for the pair: all 8 banks (2 generations x 4 tags).
            ps = {}
            for t in (0, 1):
                for o in range(NO):
                    ps[(t, o)] = pspool.tile(
                        [P, ON], fp32, tag=f"ps{o}", name=f"ps_{t}_{o}"
                    )

            # PE warm-up: garbage matmuls on zeroed scratch keep the PE busy
            # through the HAM activity window so the clock-gate opens to
            # 2.4 GHz early. A bank may only take warm-up matmuls BEFORE its
            # first real (start=True) accumulation begins, so the fills sit
            # before the first real matmul of t0 resp. t1.
            def warm(bank, n):
                for _ in range(n):
                    nc.tensor.matmul(
                        bank[:, :P], scr[:, :P], scr[:, :P], start=True, stop=True
                    )

            warm(ps[(0, 0)], N_WARM)

            # ---- pair phase: t=0,1 interleaved, k outermost ----
            # o-outer / t-inner within each k so the k=0 matmuls consume the
            # w0 pieces in their DMA arrival order.
            for k in range(KT):
                for o in range(NO):
                    a, b = wsl(k, o)
                    for t in (0, 1):
                        if k == 0 and o == 0 and t == 1:
                            warm(ps[(1, 0)], N_WARM2)
                        lhsT = x_tiles[t][:, k * P : (k + 1) * P]
                        nc.tensor.matmul(
                            ps[(t, o)][:],
                            lhsT,
                            wall[:, a:b],
                            start=(k == 0),
                            stop=(k == KT - 1),
                        )
            for t in (0, 1):
                o_t = opool.tile([P, D_OUT], fp32, tag="o", name=f"o_{t}")
                for o in range(NO):
                    nc.vector.tensor_copy(o_t[:, o * ON : (o + 1) * ON], ps[(t, o)][:])
                nc.sync.dma_start(
                    out[t * P : (t + 1) * P, : D_OUT // 2], o_t[:, : D_OUT // 2]
                )
                nc.sync.dma_start(
                    out[t * P : (t + 1) * P, D_OUT // 2 :], o_t[:, D_OUT // 2 :]
                )

            # ---- steady phase: t=2..14, one tile at a time ----
            for t in range(2, TT - 1):
                if t not in x_tiles:
                    x_tiles[t] = xpool.tile([P, KT * P], idt, tag="x", name=f"x_{t}")
                    nc.scalar.dma_start(x_tiles[t][:], xt[t])
                pst = [
                    pspool.tile([P, ON], fp32, tag=f"ps{o}", name=f"ps_{t}_{o}")
                    for o in range(NO)
                ]
                for k in range(KT):
                    lhsT = x_tiles[t][:, k * P : (k + 1) * P]
                    for o in range(NO):
                        a, b = wsl(k, o)
                        nc.tensor.matmul(
                            pst[o][:],
                            lhsT,
                            wall[:, a:b],
                            start=(k == 0),
                            stop=(k == KT - 1),
                        )
                o_t = opool.tile([P, D_OUT], fp32, tag="o", name=f"o_{t}")
                for o in range(NO):
                    nc.vector.tensor_copy(o_t[:, o * ON : (o + 1) * ON], pst[o][:])
                nc.sync.dma_start(
                    out[t * P : (t + 1) * P, : D_OUT // 2], o_t[:, : D_OUT // 2]
                )
                nc.sync.dma_start(
                    out[t * P : (t + 1) * P, D_OUT // 2 :], o_t[:, D_OUT // 2 :]
                )

            # ---- tail tile t=15: o-outer so banks complete progressively ----
            t = TT - 1
            x_tiles[t] = xpool.tile([P, KT * P], idt, tag="x", name=f"x_{t}")
            nc.scalar.dma_start(x_tiles[t][:], xt[t])
            ps15 = [
                pspool.tile([P, ON], fp32, tag=f"ps{o}", name=f"ps_{t}_{o}")
                for o in range(NO)
            ]
            o_t = opool.tile([P, D_OUT], fp32, tag="o", name=f"o_{t}")
            out_eng = [nc.sync, nc.scalar, nc.gpsimd, nc.sync]
            for o in range(NO):
                for k in range(KT):
                    lhsT = x_tiles[t][:, k * P : (k + 1) * P]
                    a, b = wsl(k, o)
                    nc.tensor.matmul(
                        ps15[o][:],
                        lhsT,
                        wall[:, a:b],
                        start=(k == 0),
                        stop=(k == KT - 1),
                    )
                nc.vector.tensor_copy(o_t[:, o * ON : (o + 1) * ON], ps15[o][:])
                out_eng[o].dma_start(
                    out[t * P : (t + 1) * P, o * ON : (o + 1) * ON],
                    o_t[:, o * ON : (o + 1) * ON],
                )

    nc.compile()
    return nc


def _get_nc():
    if IN_DTYPE not in _NC:
        _NC[IN_DTYPE] = _build_nc()
    return _NC[IN_DTYPE]


def _prep_inputs(inputs):
    x = np.ascontiguousarray(np.asarray(inputs["x"], dtype=np.float32))
    W = np.asarray(inputs["W"], dtype=np.float32)
    lora_a = np.asarray(inputs["lora_a"], dtype=np.float32)
    lora_b = np.asarray(inputs["lora_b"], dtype=np.float32)
    scalings = np.asarray(inputs["scalings"], dtype=np.float32)
    idt = _np_in_dtype()

    # Fold LoRA into the transposed effective weight per adapter:
    # Weff.T = W.T + s * A.T @ B.T  -> [d_in, d_out],
    # laid out as [P, KT*D_OUT] with wt[p, k*D_OUT+o] = Weff.T[k*128+p, o].
    wts = []
    for g in range(N_ADAPTERS):
        weff_t = W.T + scalings[g] * (lora_a[g].T @ lora_b[g].T)
        wts.append(
            np.ascontiguousarray(
                weff_t.reshape(KT, P, D_OUT).transpose(1, 0, 2).astype(idt)
            ).reshape(P, KT * D_OUT)
        )

    in_maps = []
    for c in range(N_CORES):
        xs = x[c * T_LOC : (c + 1) * T_LOC]  # [2048 tok, 2048 d]
        # [t, j, k, p] -> [t, p, k, j] -> [TT, 128, KT*128]
        xtl = np.ascontiguousarray(
            xs.reshape(TT, P, KT, P).transpose(0, 3, 2, 1).astype(idt)
        ).reshape(TT, P, KT * P)
        in_maps.append({"xt": xtl, "wt": wts[c * T_LOC // (TOKENS // N_ADAPTERS)]})
    return in_maps


def _run(inputs, trace=False, **kwargs):
    from concourse.bass_utils import run_bass_kernel_spmd

    nc = _get_nc()
    in_maps = _prep_inputs(inputs)
    res = run_bass_kernel_spmd(
        nc, in_maps, core_ids=list(range(N_CORES)), trace=trace, **kwargs
    )
    out = np.concatenate([r["out"] for r in res.results], axis=0)
    return out, res


def kernel(**inputs):
    out, _ = _run(inputs, trace=False)
    return out
